# revision 6
# baseline (speedup 1.0000x reference)
"""Bass/Tile kernel for nn_MCA (multi-head cross-attention), 8-core SPMD.

Sharding: batch B(4) x head-group(2) -> 8 cores. Core c handles batch
b = c//2 and heads [g*8, (g+1)*8) where g = c%2. Each core computes a
partial output (T, C) = y_g @ Wu[:, g-cols].T; host sums the two
head-group partials per batch and adds bu.

Single fused pipeline (all matmuls bf16 -> fp32 PSUM, uniform
(128,128) PE tile mode so the array never drains on a mode switch):

  prologue: project qT(qb0) and kT/v for T-blocks 0-1.
  main loop over 256 groups (qb, hp, ktp, h): software-pipelined
    S(g+1) matmuls -> exp(g) on ACT [128,2,512] -> V(g-1) matmuls,
    with the remaining projections (kT/v blocks 2-3, qT(qb+1)) and
    the output projection of the previous q-block woven into the PE
    stream as paced filler, so the projection work hides under the
    ACT-bound exp stream instead of running as serial phases.
  S matmuls are zero-padded to K=128: kTz stores each head's 64 k-dims
  on its own partition half with zeros on the other half, so the
  moving operand can be the full 128-partition qT slice (the other
  head's q values are multiplied by zeros).
  Rowsum via the ones-column trick (V stationary is [128, 65]); per
  head: reciprocal_approx_fast on the rowsum row, gpsimd partition
  broadcast, DVE multiply (PSUM read) into yT.
"""

import os
from contextlib import ExitStack

import numpy as np

_PROGRAM_CACHE = {}


def _imports():
    import concourse.bass as bass
    import concourse.tile as tile
    from concourse import bacc, mybir
    from concourse.bass_utils import run_bass_kernel_spmd

    return bass, tile, bacc, mybir, run_bass_kernel_spmd


def build_program(T=2048, C=1024, HLOC=8, n_cores=8):
    """Build + compile the per-core Tile program (SPMD; same for all cores)."""
    bass, tile, bacc, mybir, _ = _imports()
    BF16 = mybir.dt.bfloat16
    F32 = mybir.dt.float32
    AF = mybir.ActivationFunctionType
    ALU = mybir.AluOpType

    hd = 64
    DG = HLOC * hd            # head-group feature dim (512)
    P = 128
    KT = C // P               # contraction tiles for projections (8)
    MT = DG // P              # d-tiles (4)
    NBLK = 512                # t-block width for projections / q-blocks
    NB = T // NBLK            # 4
    KTT = T // P              # key tiles in attention (16)
    HP = MT                   # head pairs == d-tiles
    scale = 1.0 / np.sqrt(C)

    nc = bacc.Bacc("TRN2", target_bir_lowering=False, debug=False,
                   num_devices=n_cores)

    xqT = nc.dram_tensor("xqT", [C, T], BF16, kind="ExternalInput").ap()
    xkT = nc.dram_tensor("xkT", [C, T], BF16, kind="ExternalInput").ap()
    wqT = nc.dram_tensor("wqT", [C, DG], BF16, kind="ExternalInput").ap()
    wkT = nc.dram_tensor("wkT", [C, DG], BF16, kind="ExternalInput").ap()
    wvT = nc.dram_tensor("wvT", [C, DG], BF16, kind="ExternalInput").ap()
    wuT = nc.dram_tensor("wuT", [DG, C], BF16, kind="ExternalInput").ap()
    bq = nc.dram_tensor("bq", [P, MT], F32, kind="ExternalInput").ap()
    bk = nc.dram_tensor("bk", [P, MT], F32, kind="ExternalInput").ap()
    bv = nc.dram_tensor("bv", [1, DG], F32, kind="ExternalInput").ap()
    out = nc.dram_tensor("out", [T, C], F32, kind="ExternalOutput").ap()

    with tile.TileContext(nc) as tc, ExitStack() as ctx:
        const = ctx.enter_context(tc.tile_pool(name="const", bufs=1))
        persist = ctx.enter_context(tc.tile_pool(name="persist", bufs=1))
        xpool = ctx.enter_context(tc.tile_pool(name="xpool", bufs=2))
        # PSUM budget (8 banks): S 2x[128,2,512]=4, o 2x[65,512]=2,
        # proj 2x[128,512]=2
        ps_s = ctx.enter_context(tc.tile_pool(name="ps_s", bufs=2,
                                              space="PSUM"))
        ps_o = ctx.enter_context(tc.tile_pool(name="ps_o", bufs=2,
                                              space="PSUM"))
        ps_p = ctx.enter_context(tc.tile_pool(name="ps_p", bufs=2,
                                              space="PSUM"))
        ppool = ctx.enter_context(tc.tile_pool(name="ppool", bufs=3))
        epil = ctx.enter_context(tc.tile_pool(name="epil", bufs=4))
        outp = ctx.enter_context(tc.tile_pool(name="outp", bufs=3))

        # ---- constants / weights resident in SBUF ----
        wq_sb = persist.tile([P, KT, DG], BF16)
        wk_sb = persist.tile([P, KT, DG], BF16)
        wv_sb = persist.tile([P, KT, DG], BF16)
        wu_sb = persist.tile([P, MT, C], BF16)
        nc.sync.dma_start(out=wq_sb[:], in_=wqT.rearrange("(k p) d -> p k d", p=P))
        nc.sync.dma_start(out=wk_sb[:], in_=wkT.rearrange("(k p) d -> p k d", p=P))
        nc.sync.dma_start(out=wv_sb[:], in_=wvT.rearrange("(k p) d -> p k d", p=P))
        nc.sync.dma_start(out=wu_sb[:], in_=wuT.rearrange("(k p) d -> p k d", p=P))

        bq_sb = const.tile([P, MT], F32)
        bk_sb = const.tile([P, MT], F32)
        nc.sync.dma_start(out=bq_sb[:], in_=bq)
        nc.sync.dma_start(out=bk_sb[:], in_=bk)
        # bv broadcast to all 128 partitions once (used during v eviction)
        bv_bc = const.tile([P, DG], F32)
        nc.sync.dma_start(out=bv_bc[:], in_=bv.partition_broadcast(P))

        # persistent activations
        qT_sb = persist.tile([P, MT, T], BF16)
        # kTz: one [128, T] slice per head, the head's 64 k-dims on its
        # partition half ((h%2)*64) and zeros on the other half, so S
        # matmuls run with K=128 (uniform tile mode, no PE drains).
        kTz_sb = persist.tile([P, HLOC, T], BF16)
        v_aug = persist.tile([P, KTT, HLOC * (hd + 1)], BF16)
        yT_sb = persist.tile([P, MT, T], BF16)

        # zeros for the padded halves of kTz; ones column for the rowsum
        # trick (v eviction overwrites the 64-wide head slices, col 64
        # stays 1.0)
        nc.gpsimd.memset(kTz_sb[:], 0.0)
        nc.gpsimd.memset(v_aug[:], 1.0)

        xq_r = xqT.rearrange("(k p) t -> p k t", p=P)
        xk_r = xkT.rearrange("(k p) t -> p k t", p=P)

        # ---------- projection emitters (called from prologue/filler) ----
        xq_tiles = {}
        xk_tiles = {}
        proj_ps = {}

        def load_xq(nt):
            if nt not in xq_tiles:
                t = xpool.tile([P, KT, NBLK], BF16, tag="xq", name="xq_t")
                nc.sync.dma_start(
                    out=t[:], in_=xq_r[:, :, nt * NBLK:(nt + 1) * NBLK])
                xq_tiles[nt] = t
            return xq_tiles[nt]

        def load_xk(nt):
            if nt not in xk_tiles:
                t = xpool.tile([P, KT, NBLK], BF16, tag="xk", name="xk_t")
                nc.sync.dma_start(
                    out=t[:], in_=xk_r[:, :, nt * NBLK:(nt + 1) * NBLK])
                xk_tiles[nt] = t
            return xk_tiles[nt]

        def emit_qT(nt, m, half):
            """Half of one Q-projection output tile (4 matmuls); the
            second half evicts with bias + 1/sqrt(C) scale."""
            tsl = slice(nt * NBLK, (nt + 1) * NBLK)
            xq_t = load_xq(nt)
            msl = slice(m * P, (m + 1) * P)
            if half == 0:
                proj_ps[("q", nt, m)] = ps_p.tile([P, NBLK], F32, tag="p", name="psq")
            ps = proj_ps[("q", nt, m)]
            for k in range(4 * half, 4 * half + 4):
                nc.tensor.matmul(ps[:], wq_sb[:, k, msl], xq_t[:, k, :],
                                 start=(k == 0), stop=(k == KT - 1))
            if half == 1:
                del proj_ps[("q", nt, m)]
                nc.vector.tensor_scalar(
                    out=qT_sb[:, m, tsl], in0=ps[:],
                    scalar1=bq_sb[:, m:m + 1], scalar2=scale,
                    op0=ALU.add, op1=ALU.mult)

        def emit_kT(nt, m, half):
            """Half of one K-projection d-pair tile; the second half
            evicts into the two per-head zero-padded kTz slices."""
            tsl = slice(nt * NBLK, (nt + 1) * NBLK)
            xk_t = load_xk(nt)
            msl = slice(m * P, (m + 1) * P)
            if half == 0:
                proj_ps[("k", nt, m)] = ps_p.tile([P, NBLK], F32, tag="p", name="psk")
            ps = proj_ps[("k", nt, m)]
            for k in range(4 * half, 4 * half + 4):
                nc.tensor.matmul(ps[:], wk_sb[:, k, msl], xk_t[:, k, :],
                                 start=(k == 0), stop=(k == KT - 1))
            if half == 1:
                del proj_ps[("k", nt, m)]
                # head 2m -> partitions 0:64 of slice 2m; head 2m+1 ->
                # partitions 64:128 of slice 2m+1
                nc.vector.tensor_scalar(
                    out=kTz_sb[0:hd, 2 * m, tsl], in0=ps[0:hd, :],
                    scalar1=bk_sb[0:hd, m:m + 1], scalar2=None, op0=ALU.add)
                nc.vector.tensor_scalar(
                    out=kTz_sb[hd:P, 2 * m + 1, tsl], in0=ps[hd:P, :],
                    scalar1=bk_sb[hd:P, m:m + 1], scalar2=None, op0=ALU.add)

        def emit_v(nt, m, half):
            """Half of one V-projection t-subtile (all DG cols)."""
            tidx = nt * (NBLK // P) + m
            msl = slice(m * P, (m + 1) * P)
            xk_t = load_xk(nt)
            if half == 0:
                proj_ps[("v", nt, m)] = ps_p.tile([P, DG], F32, tag="p", name="psv")
            ps = proj_ps[("v", nt, m)]
            for k in range(4 * half, 4 * half + 4):
                nc.tensor.matmul(ps[:], xk_t[:, k, msl], wv_sb[:, k, :],
                                 start=(k == 0), stop=(k == KT - 1))
            if half == 1:
                del proj_ps[("v", nt, m)]
                nc.vector.tensor_add(
                    v_aug[:, tidx].rearrange("p (h e) -> p h e", e=hd + 1)[:, :, 0:hd],
                    ps[:].rearrange("p (h e) -> p h e", e=hd),
                    bv_bc[:].rearrange("p (h e) -> p h e", e=hd))

        def emit_outproj(qt, jt):
            """One [128 t, 512 c] tile of the output projection."""
            qsl = slice(qt * P, (qt + 1) * P)
            jsl = slice(jt * NBLK, (jt + 1) * NBLK)
            ps = ps_p.tile([P, NBLK], F32, tag="p")
            for dt in range(MT):
                nc.tensor.matmul(ps[:], yT_sb[:, dt, qsl],
                                 wu_sb[:, dt, jsl],
                                 start=(dt == 0), stop=(dt == MT - 1))
            o_sb = outp.tile([P, NBLK], F32, tag="osb")
            nc.vector.tensor_copy(o_sb[:], ps[:])
            nc.sync.dma_start(out=out[qsl, jsl], in_=o_sb[:])

        # ---- filler stream: (n_matmuls, thunk) pairs, paced by budget ----
        filler = []

        def drain_filler(mm_budget):
            while mm_budget > 0 and filler:
                n_mm, fn = filler.pop(0)
                fn()
                mm_budget -= n_mm

        def queue_kv_block(nt):
            for m in range(MT):
                for half in range(2):
                    filler.append(
                        (4, lambda nt=nt, m=m, h=half: emit_kT(nt, m, h)))
            for m in range(NBLK // P):
                for half in range(2):
                    filler.append(
                        (4, lambda nt=nt, m=m, h=half: emit_v(nt, m, h)))

        def queue_qT(nt):
            for m in range(MT):
                for half in range(2):
                    filler.append(
                        (4, lambda nt=nt, m=m, h=half: emit_qT(nt, m, h)))

        def queue_outproj(qb):
            for qt in range(qb * NBLK // P, (qb + 1) * NBLK // P):
                for jt in range(C // NBLK):
                    filler.append(
                        (4, lambda qt=qt, jt=jt: emit_outproj(qt, jt)))

        # ---- prologue: qT(qb0), kT/v for T-blocks 0-1 ----
        for m in range(MT):
            for half in range(2):
                emit_qT(0, m, half)
        for nt in range(NB):
            for m in range(MT):
                for half in range(2):
                    emit_kT(nt, m, half)
            for m in range(NBLK // P):
                for half in range(2):
                    emit_v(nt, m, half)

        # ---- main attention loop ----
        # group g = (qb, hp, ktp, h2): S for 2 k-tiles of one head.
        groups = []
        for qb in range(NB):
            for hp in range(HP):
                for ktp in range(KTT // 2):
                    for h2 in range(2):
                        groups.append((qb, hp, ktp, h2))
        NG = len(groups)

        def emit_S(g):
            qb, hp, ktp, h2 = groups[g]
            h = 2 * hp + h2
            qsl = slice(qb * NBLK, (qb + 1) * NBLK)
            s = ps_s.tile([P, 2, NBLK], F32, tag="s", name="s_ps")
            for j in range(2):
                kt = 2 * ktp + j
                ksl = slice(kt * P, (kt + 1) * P)
                nc.tensor.matmul(s[:, j, :], kTz_sb[:, h, ksl],
                                 qT_sb[:, hp, qsl], start=True, stop=True)
            return s

        def emit_exp(g, s):
            p = ppool.tile([P, 2, NBLK], BF16, tag="p", name="p_sb")
            nc.scalar.activation(p[:], s[:], AF.Exp)
            return p

        o_tiles = {}   # (qb, h) -> psum tile accumulating [65, NBLK]

        def emit_norm(qb, hp, h2, o_t):
            """yT[hrow, hp, qsl] = o[0:64] * 1/rowsum (row 64)."""
            qsl = slice(qb * NBLK, (qb + 1) * NBLK)
            hrow = slice(0, hd) if h2 == 0 else slice(hd, P)
            o_sb = epil.tile([hd + 1, NBLK], F32, tag="o_sb")
            nc.vector.tensor_copy(o_sb[:], o_t[:])
            recip = epil.tile([1, NBLK], F32, tag="recip")
            nc.vector.reciprocal(recip[:], o_sb[hd:hd + 1, :])
            bcast = epil.tile([hd, NBLK], F32, tag="bcast")
            nc.gpsimd.partition_broadcast(bcast[:], recip[:])
            nc.vector.tensor_mul(yT_sb[hrow, hp, qsl], o_sb[0:hd, :],
                                 bcast[:])

        def emit_V(g, p):
            qb, hp, ktp, h2 = groups[g]
            h = 2 * hp + h2
            key = (qb, h)
            if ktp == 0:
                o_tiles[key] = ps_o.tile([hd + 1, NBLK], F32, tag="o", name="o_ps")
            o_t = o_tiles[key]
            for j in range(2):
                kt = 2 * ktp + j
                nc.tensor.matmul(
                    o_t[:], v_aug[:, kt, h * (hd + 1):(h + 1) * (hd + 1)],
                    p[:, j, :], start=(kt == 0), stop=(kt == KTT - 1))
            if ktp == KTT // 2 - 1:
                emit_norm(qb, hp, h2, o_tiles.pop(key))

        # software pipeline: S(g+1) ahead of exp(g) ahead of V(g-1)
        s_tiles = {0: emit_S(0)}
        p_tiles = {}
        for g in range(NG):
            qb, hp, ktp, h2 = groups[g]
            if hp == 0 and ktp == 0 and h2 == 0:
                # q-block boundary: queue next q-block's Q projection and
                # the finished previous q-block's output projection
                if qb + 1 < NB:
                    queue_qT(qb + 1)
                if qb >= 1:
                    queue_outproj(qb - 1)
            if g + 1 < NG:
                s_tiles[g + 1] = emit_S(g + 1)
            p_tiles[g] = emit_exp(g, s_tiles.pop(g))
            if g - 1 >= 0:
                emit_V(g - 1, p_tiles.pop(g - 1))
            # pace the filler: aggressive while kT/v blocks are pending
            # (the k-sweep needs them), then ~1-2 matmuls per group so the
            # exp stream is never starved of S tiles
            drain_filler(8 if g < 24 else 2)
        emit_V(NG - 1, p_tiles.pop(NG - 1))

        # ---- epilogue: last q-block's output projection + leftovers ----
        queue_outproj(NB - 1)
        drain_filler(10 ** 9)

    nc.compile()
    return nc


def _get_program():
    key = "main"
    if key not in _PROGRAM_CACHE:
        _PROGRAM_CACHE[key] = build_program()
    return _PROGRAM_CACHE[key]


def make_in_maps(x1, x2, Wq, bq, Wk, bk, Wv, bv, Wu, bu, n_cores=8):
    import ml_dtypes
    bf16 = ml_dtypes.bfloat16
    T, B, C = x1.shape
    H = 16
    DG = C // 2  # head-group feature dim (8 heads x 64)
    x1 = np.asarray(x1, np.float32)
    x2 = np.asarray(x2, np.float32)
    in_maps = []
    for core in range(n_cores):
        b, g = core // 2, core % 2
        gs = slice(g * DG, (g + 1) * DG)
        in_maps.append({
            "xqT": np.ascontiguousarray(x1[:, b, :].T).astype(bf16),
            "xkT": np.ascontiguousarray(x2[:, b, :].T).astype(bf16),
            "wqT": np.ascontiguousarray(np.asarray(Wq)[gs, :].T).astype(bf16),
            "wkT": np.ascontiguousarray(np.asarray(Wk)[gs, :].T).astype(bf16),
            "wvT": np.ascontiguousarray(np.asarray(Wv)[gs, :].T).astype(bf16),
            "wuT": np.ascontiguousarray(np.asarray(Wu)[:, gs].T).astype(bf16),
            "bq": np.ascontiguousarray(
                np.asarray(bq, np.float32)[gs].reshape(-1, 128).T),
            "bk": np.ascontiguousarray(
                np.asarray(bk, np.float32)[gs].reshape(-1, 128).T),
            "bv": np.asarray(bv, np.float32)[gs].reshape(1, DG),
        })
    return in_maps


def kernel(x1, x2, Wq, bq, Wk, bk, Wv, bv, Wu, bu, _results_hook=None):
    _, _, _, _, run_bass_kernel_spmd = _imports()
    T, B, C = x1.shape
    nc = _get_program()
    in_maps = make_in_maps(x1, x2, Wq, bq, Wk, bk, Wv, bv, Wu, bu)
    br = run_bass_kernel_spmd(nc, in_maps, list(range(8)))
    if _results_hook is not None:
        _results_hook(br)
    outs = [np.asarray(r["out"], np.float32) for r in br.results]
    bu = np.asarray(bu, np.float32)
    full = np.stack([outs[2 * b] + outs[2 * b + 1] for b in range(B)], axis=0)
    full += bu.reshape(1, 1, -1)
    return full.astype(np.float32)


# revision 19
# speedup vs baseline: 1.1422x; 1.1422x over previous
"""Bass/Tile kernel for nn_MCA (multi-head cross-attention), 8-core SPMD.

Sharding: batch B(4) x head-group(2) -> 8 cores. Core c handles batch
b = c//2 and heads [g*8, (g+1)*8) where g = c%2. Each core computes a
partial output (T, C) = y_g @ Wu[:, g-cols].T; host sums the two
head-group partials per batch and adds bu.

Single fused pipeline (all matmuls bf16 -> fp32 PSUM, uniform
(128,128) PE tile mode so the array never drains on a mode switch):

  prologue: project qT(qb0) and kT/v for T-blocks 0-1.
  main loop over 256 groups (qb, hp, ktp, h): software-pipelined
    S(g+1) matmuls -> exp(g) on ACT [128,2,512] -> V(g-1) matmuls,
    with the remaining projections (kT/v blocks 2-3, qT(qb+1)) and
    the output projection of the previous q-block woven into the PE
    stream as paced filler, so the projection work hides under the
    ACT-bound exp stream instead of running as serial phases.
  S matmuls are zero-padded to K=128: kTz stores each head's 64 k-dims
  on its own partition half with zeros on the other half, so the
  moving operand can be the full 128-partition qT slice (the other
  head's q values are multiplied by zeros).
  Rowsum via the ones-column trick (V stationary is [128, 65]); per
  head: reciprocal_approx_fast on the rowsum row, gpsimd partition
  broadcast, DVE multiply (PSUM read) into yT.
"""

import os
from contextlib import ExitStack

import numpy as np

_PROGRAM_CACHE = {}


def _imports():
    import concourse.bass as bass
    import concourse.tile as tile
    from concourse import bacc, mybir
    from concourse.bass_utils import run_bass_kernel_spmd

    return bass, tile, bacc, mybir, run_bass_kernel_spmd


def build_program(T=2048, C=1024, HLOC=8, n_cores=8):
    """Build + compile the per-core Tile program (SPMD; same for all cores)."""
    bass, tile, bacc, mybir, _ = _imports()
    BF16 = mybir.dt.bfloat16
    F32 = mybir.dt.float32
    AF = mybir.ActivationFunctionType
    ALU = mybir.AluOpType

    hd = 64
    DG = HLOC * hd            # head-group feature dim (512)
    P = 128
    KT = C // P               # contraction tiles for projections (8)
    MT = DG // P              # d-tiles (4)
    NBLK = 512                # t-block width for projections / q-blocks
    NB = T // NBLK            # 4
    KTT = T // P              # key tiles in attention (16)
    HP = MT                   # head pairs == d-tiles
    scale = 1.0 / np.sqrt(C)

    nc = bacc.Bacc("TRN2", target_bir_lowering=False, debug=False,
                   num_devices=n_cores)

    xqT = nc.dram_tensor("xqT", [C, T], BF16, kind="ExternalInput").ap()
    xkT = nc.dram_tensor("xkT", [C, T], BF16, kind="ExternalInput").ap()
    wqT = nc.dram_tensor("wqT", [C, DG], BF16, kind="ExternalInput").ap()
    wkT = nc.dram_tensor("wkT", [C, DG], BF16, kind="ExternalInput").ap()
    wvT = nc.dram_tensor("wvT", [C, DG], BF16, kind="ExternalInput").ap()
    wuT = nc.dram_tensor("wuT", [DG, C], BF16, kind="ExternalInput").ap()
    bq = nc.dram_tensor("bq", [P, MT], F32, kind="ExternalInput").ap()
    bk = nc.dram_tensor("bk", [P, MT], F32, kind="ExternalInput").ap()
    bv = nc.dram_tensor("bv", [1, DG], F32, kind="ExternalInput").ap()
    out = nc.dram_tensor("out", [T, C], F32, kind="ExternalOutput").ap()
    dbg_v = nc.dram_tensor("dbg_v", [128, T // 128, HLOC * 65], BF16,
                           kind="ExternalOutput").ap()
    dbg_y = nc.dram_tensor("dbg_y", [128, C // 256, T], BF16,
                           kind="ExternalOutput").ap()

    with tile.TileContext(nc) as tc, ExitStack() as ctx:
        const = ctx.enter_context(tc.tile_pool(name="const", bufs=1))
        persist = ctx.enter_context(tc.tile_pool(name="persist", bufs=1))
        xpool = ctx.enter_context(tc.tile_pool(name="xpool", bufs=2))
        # PSUM budget (8 banks): S 2x[128,2,512]=4, o 2x[65,512]=2,
        # proj 2x[128,512]=2
        ps_s = ctx.enter_context(tc.tile_pool(name="ps_s", bufs=2,
                                              space="PSUM"))
        ps_o = ctx.enter_context(tc.tile_pool(name="ps_o", bufs=2,
                                              space="PSUM"))
        ps_p = ctx.enter_context(tc.tile_pool(name="ps_p", bufs=2,
                                              space="PSUM"))
        ppool = ctx.enter_context(tc.tile_pool(name="ppool", bufs=3))
        epil = ctx.enter_context(tc.tile_pool(name="epil", bufs=4))
        outp = ctx.enter_context(tc.tile_pool(name="outp", bufs=3))

        # ---- constants / weights resident in SBUF ----
        wq_sb = persist.tile([P, KT, DG], BF16)
        wk_sb = persist.tile([P, KT, DG], BF16)
        wv_sb = persist.tile([P, KT, DG], BF16)
        wu_sb = persist.tile([P, MT, C], BF16)
        nc.sync.dma_start(out=wq_sb[:], in_=wqT.rearrange("(k p) d -> p k d", p=P))
        nc.sync.dma_start(out=wk_sb[:], in_=wkT.rearrange("(k p) d -> p k d", p=P))
        nc.sync.dma_start(out=wv_sb[:], in_=wvT.rearrange("(k p) d -> p k d", p=P))
        nc.sync.dma_start(out=wu_sb[:], in_=wuT.rearrange("(k p) d -> p k d", p=P))

        bq_sb = const.tile([P, MT], F32)
        bk_sb = const.tile([P, MT], F32)
        nc.sync.dma_start(out=bq_sb[:], in_=bq)
        nc.sync.dma_start(out=bk_sb[:], in_=bk)
        # bv broadcast to all 128 partitions once (used during v eviction)
        bv_bc = const.tile([P, DG], F32)
        nc.sync.dma_start(out=bv_bc[:], in_=bv.partition_broadcast(P))

        # persistent activations
        qT_sb = persist.tile([P, MT, T], BF16)
        # kTz: one [128, T] slice per head, the head's 64 k-dims on its
        # partition half ((h%2)*64) and zeros on the other half, so S
        # matmuls run with K=128 (uniform tile mode, no PE drains).
        kTz_sb = persist.tile([P, HLOC, T], BF16)
        v_aug = persist.tile([P, KTT, HLOC * (hd + 1)], BF16)
        yT_sb = persist.tile([P, MT, T], BF16)

        # zeros for the padded halves of kTz. v_aug is fully written by the
        # evictions (data chunks + ones columns) -- a prior whole-tensor
        # memset writer breaks the scheduler's read-dependency attribution
        # for later partial writers.
        nc.gpsimd.memset(kTz_sb[:], 0.0)

        xq_r = xqT.rearrange("(k p) t -> p k t", p=P)
        xk_r = xkT.rearrange("(k p) t -> p k t", p=P)

        # ---------- projection emitters (called from prologue/filler) ----
        xq_tiles = {}
        xk_tiles = {}
        proj_ps = {}

        def load_xq(nt):
            if nt not in xq_tiles:
                t = xpool.tile([P, KT, NBLK], BF16, tag="xq", name="xq_t")
                nc.sync.dma_start(
                    out=t[:], in_=xq_r[:, :, nt * NBLK:(nt + 1) * NBLK])
                xq_tiles[nt] = t
            return xq_tiles[nt]

        def load_xk(nt):
            if nt not in xk_tiles:
                t = xpool.tile([P, KT, NBLK], BF16, tag="xk", name="xk_t")
                nc.sync.dma_start(
                    out=t[:], in_=xk_r[:, :, nt * NBLK:(nt + 1) * NBLK])
                xk_tiles[nt] = t
            return xk_tiles[nt]

        def emit_qT(nt, m, half):
            """Half of one Q-projection output tile (4 matmuls); the
            second half evicts with bias + 1/sqrt(C) scale."""
            tsl = slice(nt * NBLK, (nt + 1) * NBLK)
            xq_t = load_xq(nt)
            msl = slice(m * P, (m + 1) * P)
            if half == 0:
                proj_ps[("q", nt, m)] = ps_p.tile([P, NBLK], F32, tag="p", name="psq")
            ps = proj_ps[("q", nt, m)]
            for k in range(4 * half, 4 * half + 4):
                nc.tensor.matmul(ps[:], wq_sb[:, k, msl], xq_t[:, k, :],
                                 start=(k == 0), stop=(k == KT - 1))
            if half == 1:
                del proj_ps[("q", nt, m)]
                nc.vector.tensor_scalar(
                    out=qT_sb[:, m, tsl], in0=ps[:],
                    scalar1=bq_sb[:, m:m + 1], scalar2=scale,
                    op0=ALU.add, op1=ALU.mult)

        def emit_kT(nt, m, half):
            """Half of one K-projection d-pair tile; the second half
            evicts into the two per-head zero-padded kTz slices."""
            tsl = slice(nt * NBLK, (nt + 1) * NBLK)
            xk_t = load_xk(nt)
            msl = slice(m * P, (m + 1) * P)
            if half == 0:
                proj_ps[("k", nt, m)] = ps_p.tile([P, NBLK], F32, tag="p", name="psk")
            ps = proj_ps[("k", nt, m)]
            for k in range(4 * half, 4 * half + 4):
                nc.tensor.matmul(ps[:], wk_sb[:, k, msl], xk_t[:, k, :],
                                 start=(k == 0), stop=(k == KT - 1))
            if half == 1:
                del proj_ps[("k", nt, m)]
                # head 2m -> partitions 0:64 of slice 2m; head 2m+1 ->
                # partitions 64:128 of slice 2m+1
                nc.vector.tensor_scalar(
                    out=kTz_sb[0:hd, 2 * m, tsl], in0=ps[0:hd, :],
                    scalar1=bk_sb[0:hd, m:m + 1], scalar2=None, op0=ALU.add)
                nc.vector.tensor_scalar(
                    out=kTz_sb[hd:P, 2 * m + 1, tsl], in0=ps[hd:P, :],
                    scalar1=bk_sb[hd:P, m:m + 1], scalar2=None, op0=ALU.add)

        def emit_v(nt, m, half):
            """Half of one V-projection t-subtile (all DG cols)."""
            tidx = nt * (NBLK // P) + m
            msl = slice(m * P, (m + 1) * P)
            xk_t = load_xk(nt)
            if half == 0:
                proj_ps[("v", nt, m)] = ps_p.tile([P, DG], F32, tag="p", name="psv")
            ps = proj_ps[("v", nt, m)]
            for k in range(4 * half, 4 * half + 4):
                nc.tensor.matmul(ps[:], xk_t[:, k, msl], wv_sb[:, k, :],
                                 start=(k == 0), stop=(k == KT - 1))
            if half == 1:
                del proj_ps[("v", nt, m)]
                v_row = v_aug[:, tidx]
                for h in range(HLOC):
                    nc.vector.tensor_add(
                        v_row[:, h * (hd + 1):h * (hd + 1) + hd],
                        ps[:, h * hd:(h + 1) * hd],
                        bv_bc[:, h * hd:(h + 1) * hd])
                # rowsum-trick ones columns, written here rather than by a
                # whole-tensor memset (see note at kTz memset)
                ones_view = v_row.rearrange("p (h e) -> p h e", e=hd + 1)[:, :, hd:hd + 1]
                nc.vector.memset(ones_view, 1.0)

        def emit_outproj(qt, jt):
            """One [128 t, 512 c] tile of the output projection."""
            qsl = slice(qt * P, (qt + 1) * P)
            jsl = slice(jt * NBLK, (jt + 1) * NBLK)
            ps = ps_p.tile([P, NBLK], F32, tag="p")
            for dt in range(MT):
                nc.tensor.matmul(ps[:], yT_sb[:, dt, qsl],
                                 wu_sb[:, dt, jsl],
                                 start=(dt == 0), stop=(dt == MT - 1))
            o_sb = outp.tile([P, NBLK], F32, tag="osb")
            nc.vector.tensor_copy(o_sb[:], ps[:])
            nc.sync.dma_start(out=out[qsl, jsl], in_=o_sb[:])

        # ---- filler stream: (n_matmuls, thunk) pairs, paced by budget ----
        filler = []

        def drain_filler(mm_budget):
            while mm_budget > 0 and filler:
                n_mm, fn = filler.pop(0)
                fn()
                mm_budget -= n_mm

        def queue_kv_block(nt):
            for m in range(MT):
                for half in range(2):
                    filler.append(
                        (4, lambda nt=nt, m=m, h=half: emit_kT(nt, m, h)))
            for m in range(NBLK // P):
                for half in range(2):
                    filler.append(
                        (4, lambda nt=nt, m=m, h=half: emit_v(nt, m, h)))

        def queue_qT(nt):
            for m in range(MT):
                for half in range(2):
                    filler.append(
                        (4, lambda nt=nt, m=m, h=half: emit_qT(nt, m, h)))

        def queue_outproj(qb):
            for qt in range(qb * NBLK // P, (qb + 1) * NBLK // P):
                for jt in range(C // NBLK):
                    filler.append(
                        (4, lambda qt=qt, jt=jt: emit_outproj(qt, jt)))

        # ---- prologue: qT(qb0), kT/v for T-blocks 0-1 ----
        for m in range(MT):
            for half in range(2):
                emit_qT(0, m, half)
        for nt in range(NB):
            for m in range(MT):
                for half in range(2):
                    emit_kT(nt, m, half)
            if nt < 2:
                for m in range(NBLK // P):
                    for half in range(2):
                        emit_v(nt, m, half)
        # v blocks 2-3 as filler (bisect: does v_aug race?)
        for nt in range(2, NB):
            for m in range(NBLK // P):
                for half in range(2):
                    filler.append(
                        (4, lambda nt=nt, m=m, h=half: emit_v(nt, m, h)))

        # ---- main attention loop ----
        # group g = (qb, hp, ktp, h2): S for 2 k-tiles of one head.
        groups = []
        for qb in range(NB):
            for hp in range(HP):
                for ktp in range(KTT // 2):
                    for h2 in range(2):
                        groups.append((qb, hp, ktp, h2))
        NG = len(groups)

        def emit_S(g):
            qb, hp, ktp, h2 = groups[g]
            h = 2 * hp + h2
            qsl = slice(qb * NBLK, (qb + 1) * NBLK)
            s = ps_s.tile([P, 2, NBLK], F32, tag="s", name="s_ps")
            for j in range(2):
                kt = 2 * ktp + j
                ksl = slice(kt * P, (kt + 1) * P)
                nc.tensor.matmul(s[:, j, :], kTz_sb[:, h, ksl],
                                 qT_sb[:, hp, qsl], start=True, stop=True)
            return s

        def emit_exp(g, s):
            p = ppool.tile([P, 2, NBLK], BF16, tag="p", name="p_sb")
            nc.scalar.activation(p[:], s[:], AF.Exp)
            return p

        o_tiles = {}   # (qb, h) -> psum tile accumulating [65, NBLK]
        norm_sbs = {}  # (qb, h) -> o_sb copy awaiting the batched recip
        rs_tiles = {}  # qb -> [HLOC, NBLK] gathered rowsums

        def emit_norm_copy(qb, h, o_t):
            """Evict o psum; gather the rowsum row into the per-qb batch
            (via DMA so the DVE queue stays clean)."""
            if qb not in rs_tiles:
                rs_tiles[qb] = epil.tile([HLOC, NBLK], F32, tag="rs",
                                         bufs=2, name="rs_all")
            o_sb = epil.tile([hd + 1, NBLK], F32, tag="o_sb", bufs=10,
                             name="o_sb")
            nc.vector.tensor_copy(o_sb[:], o_t[:])
            nc.sync.dma_start(out=rs_tiles[qb][h:h + 1, :],
                              in_=o_sb[hd:hd + 1, :])
            norm_sbs[(qb, h)] = o_sb

        def emit_norm_finish(qb):
            """One batched reciprocal for all 8 heads, then per-head
            broadcast + multiply into yT."""
            qsl = slice(qb * NBLK, (qb + 1) * NBLK)
            recip_t = epil.tile([HLOC, NBLK], F32, tag="recip", bufs=2,
                                name="recip_t")
            nc.vector.reciprocal(recip_t[:], rs_tiles.pop(qb)[:])
            for h in range(HLOC):
                hp, h2 = h // 2, h % 2
                hrow = slice(0, hd) if h2 == 0 else slice(hd, P)
                # partition_broadcast requires its source on partition 0
                rtmp = epil.tile([1, NBLK], F32, tag="rtmp", bufs=3,
                                 name="rtmp")
                nc.sync.dma_start(out=rtmp[:], in_=recip_t[h:h + 1, :])
                bcast = epil.tile([hd, NBLK], F32, tag="bcast", bufs=3,
                                  name="bcast")
                nc.gpsimd.partition_broadcast(bcast[:], rtmp[:])
                nc.vector.tensor_mul(yT_sb[hrow, hp, qsl],
                                     norm_sbs.pop((qb, h))[0:hd, :],
                                     bcast[:])

        def emit_V(g, p):
            qb, hp, ktp, h2 = groups[g]
            h = 2 * hp + h2
            key = (qb, h)
            if ktp == 0:
                o_tiles[key] = ps_o.tile([hd + 1, NBLK], F32, tag="o", name="o_ps")
            o_t = o_tiles[key]
            for j in range(2):
                kt = 2 * ktp + j
                nc.tensor.matmul(
                    o_t[:], v_aug[:, kt, h * (hd + 1):(h + 1) * (hd + 1)],
                    p[:, j, :], start=(kt == 0), stop=(kt == KTT - 1))
            if ktp == KTT // 2 - 1:
                emit_norm_copy(qb, h, o_tiles.pop(key))
                if hp == HP - 1 and h2 == 1:
                    emit_norm_finish(qb)

        # software pipeline: S(g+1) ahead of exp(g) ahead of V(g-1)
        s_tiles = {0: emit_S(0)}
        p_tiles = {}
        for g in range(NG):
            qb, hp, ktp, h2 = groups[g]
            if hp == 0 and ktp == 0 and h2 == 0:
                # q-block boundary: queue next q-block's Q projection and
                # the finished previous q-block's output projection
                if qb + 1 < NB:
                    queue_qT(qb + 1)
                if qb >= 1:
                    queue_outproj(qb - 1)
            if g + 1 < NG:
                s_tiles[g + 1] = emit_S(g + 1)
            p_tiles[g] = emit_exp(g, s_tiles.pop(g))
            if g - 1 >= 0:
                emit_V(g - 1, p_tiles.pop(g - 1))
            # pace the filler: aggressive while kT/v blocks are pending
            # (the k-sweep needs them), then ~1-2 matmuls per group so the
            # exp stream is never starved of S tiles
            drain_filler(8 if g < 24 else 2)
        emit_V(NG - 1, p_tiles.pop(NG - 1))

        # ---- epilogue: last q-block's output projection + leftovers ----
        queue_outproj(NB - 1)
        drain_filler(10 ** 9)

        # debug dumps (dbg_v/dbg_y outputs kept but written cheaply via
        # direct SBUF->DRAM DMA of the bf16 tensors cast on host if needed)
        for kt in range(KTT):
            nc.sync.dma_start(out=dbg_v[:, kt], in_=v_aug[:, kt])
        for mt in range(MT):
            nc.sync.dma_start(out=dbg_y[:, mt], in_=yT_sb[:, mt])

    nc.compile()
    return nc


def _get_program():
    key = "main"
    if key not in _PROGRAM_CACHE:
        _PROGRAM_CACHE[key] = build_program()
    return _PROGRAM_CACHE[key]


def make_in_maps(x1, x2, Wq, bq, Wk, bk, Wv, bv, Wu, bu, n_cores=8):
    import ml_dtypes
    bf16 = ml_dtypes.bfloat16
    T, B, C = x1.shape
    H = 16
    DG = C // 2  # head-group feature dim (8 heads x 64)
    x1 = np.asarray(x1, np.float32)
    x2 = np.asarray(x2, np.float32)
    in_maps = []
    for core in range(n_cores):
        b, g = core // 2, core % 2
        gs = slice(g * DG, (g + 1) * DG)
        in_maps.append({
            "xqT": np.ascontiguousarray(x1[:, b, :].T).astype(bf16),
            "xkT": np.ascontiguousarray(x2[:, b, :].T).astype(bf16),
            "wqT": np.ascontiguousarray(np.asarray(Wq)[gs, :].T).astype(bf16),
            "wkT": np.ascontiguousarray(np.asarray(Wk)[gs, :].T).astype(bf16),
            "wvT": np.ascontiguousarray(np.asarray(Wv)[gs, :].T).astype(bf16),
            "wuT": np.ascontiguousarray(np.asarray(Wu)[:, gs].T).astype(bf16),
            "bq": np.ascontiguousarray(
                np.asarray(bq, np.float32)[gs].reshape(-1, 128).T),
            "bk": np.ascontiguousarray(
                np.asarray(bk, np.float32)[gs].reshape(-1, 128).T),
            "bv": np.asarray(bv, np.float32)[gs].reshape(1, DG),
        })
    return in_maps


def kernel(x1, x2, Wq, bq, Wk, bk, Wv, bv, Wu, bu, _results_hook=None):
    _, _, _, _, run_bass_kernel_spmd = _imports()
    T, B, C = x1.shape
    nc = _get_program()
    in_maps = make_in_maps(x1, x2, Wq, bq, Wk, bk, Wv, bv, Wu, bu)
    br = run_bass_kernel_spmd(nc, in_maps, list(range(8)))
    if _results_hook is not None:
        _results_hook(br)
    outs = [np.asarray(r["out"], np.float32) for r in br.results]
    bu = np.asarray(bu, np.float32)
    full = np.stack([outs[2 * b] + outs[2 * b + 1] for b in range(B)], axis=0)
    full += bu.reshape(1, 1, -1)
    return full.astype(np.float32)


# revision 23
# speedup vs baseline: 1.1662x; 1.0210x over previous
"""Bass/Tile kernel for nn_MCA (multi-head cross-attention), 8-core SPMD.

Sharding: batch B(4) x head-group(2) -> 8 cores. Core c handles batch
b = c//2 and heads [g*8, (g+1)*8) where g = c%2. Each core computes a
partial output (T, C) = y_g @ Wu[:, g-cols].T; host sums the two
head-group partials per batch and adds bu.

Single fused pipeline (all matmuls bf16 -> fp32 PSUM, uniform
(128,128) PE tile mode so the array never drains on a mode switch):

  prologue: project qT(qb0) and kT/v for T-blocks 0-1.
  main loop over 256 groups (qb, hp, ktp, h): software-pipelined
    S(g+1) matmuls -> exp(g) on ACT [128,2,512] -> V(g-1) matmuls,
    with the remaining projections (kT/v blocks 2-3, qT(qb+1)) and
    the output projection of the previous q-block woven into the PE
    stream as paced filler, so the projection work hides under the
    ACT-bound exp stream instead of running as serial phases.
  S matmuls are zero-padded to K=128: kTz stores each head's 64 k-dims
  on its own partition half with zeros on the other half, so the
  moving operand can be the full 128-partition qT slice (the other
  head's q values are multiplied by zeros).
  Rowsum via the ones-column trick (V stationary is [128, 65]); per
  head: reciprocal_approx_fast on the rowsum row, gpsimd partition
  broadcast, DVE multiply (PSUM read) into yT.
"""

import os
from contextlib import ExitStack

import numpy as np

_PROGRAM_CACHE = {}


def _imports():
    import concourse.bass as bass
    import concourse.tile as tile
    from concourse import bacc, mybir
    from concourse.bass_utils import run_bass_kernel_spmd

    return bass, tile, bacc, mybir, run_bass_kernel_spmd


def build_program(T=2048, C=1024, HLOC=8, n_cores=8):
    """Build + compile the per-core Tile program (SPMD; same for all cores)."""
    bass, tile, bacc, mybir, _ = _imports()
    BF16 = mybir.dt.bfloat16
    F32 = mybir.dt.float32
    AF = mybir.ActivationFunctionType
    ALU = mybir.AluOpType

    hd = 64
    DG = HLOC * hd            # head-group feature dim (512)
    P = 128
    KT = C // P               # contraction tiles for projections (8)
    MT = DG // P              # d-tiles (4)
    NBLK = 512                # t-block width for projections / q-blocks
    NB = T // NBLK            # 4
    KTT = T // P              # key tiles in attention (16)
    HP = MT                   # head pairs == d-tiles
    scale = 1.0 / np.sqrt(C)

    nc = bacc.Bacc("TRN2", target_bir_lowering=False, debug=False,
                   num_devices=n_cores)

    xqT = nc.dram_tensor("xqT", [C, T], BF16, kind="ExternalInput").ap()
    xkT = nc.dram_tensor("xkT", [C, T], BF16, kind="ExternalInput").ap()
    wqT = nc.dram_tensor("wqT", [C, DG], BF16, kind="ExternalInput").ap()
    wkT = nc.dram_tensor("wkT", [C, DG], BF16, kind="ExternalInput").ap()
    wvT = nc.dram_tensor("wvT", [C, DG], BF16, kind="ExternalInput").ap()
    wuT = nc.dram_tensor("wuT", [DG, C], BF16, kind="ExternalInput").ap()
    bq = nc.dram_tensor("bq", [P, MT], F32, kind="ExternalInput").ap()
    bk = nc.dram_tensor("bk", [P, MT], F32, kind="ExternalInput").ap()
    bv = nc.dram_tensor("bv", [1, DG], F32, kind="ExternalInput").ap()
    out = nc.dram_tensor("out", [T, C], F32, kind="ExternalOutput").ap()

    with tile.TileContext(nc) as tc, ExitStack() as ctx:
        const = ctx.enter_context(tc.tile_pool(name="const", bufs=1))
        persist = ctx.enter_context(tc.tile_pool(name="persist", bufs=1))
        xqpool = ctx.enter_context(tc.tile_pool(name="xqpool", bufs=2))
        # all 4 xk blocks stay resident: kT m1-3 slices are projected long
        # after newer blocks load (deadline-ordered filler)
        xkpool = ctx.enter_context(tc.tile_pool(name="xkpool", bufs=4))
        # PSUM budget (8 banks): S 2x[128,2,512]=4, o 2x[65,512]=2,
        # proj 2x[128,512]=2
        ps_s = ctx.enter_context(tc.tile_pool(name="ps_s", bufs=2,
                                              space="PSUM"))
        ps_o = ctx.enter_context(tc.tile_pool(name="ps_o", bufs=2,
                                              space="PSUM"))
        ps_p = ctx.enter_context(tc.tile_pool(name="ps_p", bufs=2,
                                              space="PSUM"))
        ppool = ctx.enter_context(tc.tile_pool(name="ppool", bufs=3))
        epil = ctx.enter_context(tc.tile_pool(name="epil", bufs=4))
        outp = ctx.enter_context(tc.tile_pool(name="outp", bufs=2))

        # ---- constants / weights resident in SBUF ----
        wq_sb = persist.tile([P, KT, DG], BF16)
        wk_sb = persist.tile([P, KT, DG], BF16)
        wv_sb = persist.tile([P, KT, DG], BF16)
        wu_sb = persist.tile([P, MT, C], BF16)
        bq_sb = const.tile([P, MT], F32)
        bk_sb = const.tile([P, MT], F32)
        bv_bc = const.tile([P, DG], F32)
        # DMA in need-order: Q-projection inputs first so the PE starts
        # within a few us, output-projection weights last
        nc.sync.dma_start(out=wq_sb[:], in_=wqT.rearrange("(k p) d -> p k d", p=P))
        nc.sync.dma_start(out=bq_sb[:], in_=bq)
        nc.sync.dma_start(out=wk_sb[:], in_=wkT.rearrange("(k p) d -> p k d", p=P))
        nc.sync.dma_start(out=bk_sb[:], in_=bk)
        nc.sync.dma_start(out=wv_sb[:], in_=wvT.rearrange("(k p) d -> p k d", p=P))
        nc.sync.dma_start(out=bv_bc[:], in_=bv.partition_broadcast(P))
        nc.sync.dma_start(out=wu_sb[:], in_=wuT.rearrange("(k p) d -> p k d", p=P))

        # persistent activations
        qT_sb = persist.tile([P, MT, T], BF16)
        # kTz: one [128, T] slice per head, the head's 64 k-dims on its
        # partition half ((h%2)*64) and zeros on the other half, so S
        # matmuls run with K=128 (uniform tile mode, no PE drains).
        kTz_sb = persist.tile([P, HLOC, T], BF16)
        v_aug = persist.tile([P, KTT, HLOC * (hd + 1)], BF16)
        yT_sb = persist.tile([P, MT, T], BF16)

        # zeros for the padded halves of kTz. v_aug is fully written by the
        # evictions (data chunks + ones columns) -- a prior whole-tensor
        # memset writer breaks the scheduler's read-dependency attribution
        # for later partial writers.
        nc.gpsimd.memset(kTz_sb[:], 0.0)

        xq_r = xqT.rearrange("(k p) t -> p k t", p=P)
        xk_r = xkT.rearrange("(k p) t -> p k t", p=P)

        # ---------- projection emitters (called from prologue/filler) ----
        xq_tiles = {}
        xk_tiles = {}
        proj_ps = {}

        def load_xq(nt):
            if nt not in xq_tiles:
                t = xqpool.tile([P, KT, NBLK], BF16, tag="xq", name="xq_t")
                nc.sync.dma_start(
                    out=t[:], in_=xq_r[:, :, nt * NBLK:(nt + 1) * NBLK])
                xq_tiles[nt] = t
            return xq_tiles[nt]

        def load_xk(nt):
            if nt not in xk_tiles:
                t = xkpool.tile([P, KT, NBLK], BF16, tag="xk", name="xk_t")
                nc.sync.dma_start(
                    out=t[:], in_=xk_r[:, :, nt * NBLK:(nt + 1) * NBLK])
                xk_tiles[nt] = t
            return xk_tiles[nt]

        def emit_qT(nt, m, half):
            """Half of one Q-projection output tile (4 matmuls); the
            second half evicts with bias + 1/sqrt(C) scale."""
            tsl = slice(nt * NBLK, (nt + 1) * NBLK)
            xq_t = load_xq(nt)
            msl = slice(m * P, (m + 1) * P)
            if half == 0:
                proj_ps[("q", nt, m)] = ps_p.tile([P, NBLK], F32, tag="p", name="psq")
            ps = proj_ps[("q", nt, m)]
            for k in range(4 * half, 4 * half + 4):
                nc.tensor.matmul(ps[:], wq_sb[:, k, msl], xq_t[:, k, :],
                                 start=(k == 0), stop=(k == KT - 1))
            if half == 1:
                del proj_ps[("q", nt, m)]
                nc.vector.tensor_scalar(
                    out=qT_sb[:, m, tsl], in0=ps[:],
                    scalar1=bq_sb[:, m:m + 1], scalar2=scale,
                    op0=ALU.add, op1=ALU.mult)

        def emit_kT(nt, m, half):
            """Half of one K-projection d-pair tile; the second half
            evicts into the two per-head zero-padded kTz slices."""
            tsl = slice(nt * NBLK, (nt + 1) * NBLK)
            xk_t = load_xk(nt)
            msl = slice(m * P, (m + 1) * P)
            if half == 0:
                proj_ps[("k", nt, m)] = ps_p.tile([P, NBLK], F32, tag="p", name="psk")
            ps = proj_ps[("k", nt, m)]
            for k in range(4 * half, 4 * half + 4):
                nc.tensor.matmul(ps[:], wk_sb[:, k, msl], xk_t[:, k, :],
                                 start=(k == 0), stop=(k == KT - 1))
            if half == 1:
                del proj_ps[("k", nt, m)]
                # head 2m -> partitions 0:64 of slice 2m; head 2m+1 ->
                # partitions 64:128 of slice 2m+1
                nc.vector.tensor_scalar(
                    out=kTz_sb[0:hd, 2 * m, tsl], in0=ps[0:hd, :],
                    scalar1=bk_sb[0:hd, m:m + 1], scalar2=None, op0=ALU.add)
                nc.vector.tensor_scalar(
                    out=kTz_sb[hd:P, 2 * m + 1, tsl], in0=ps[hd:P, :],
                    scalar1=bk_sb[hd:P, m:m + 1], scalar2=None, op0=ALU.add)

        def emit_v(nt, m, half):
            """Half of one V-projection t-subtile (all DG cols)."""
            tidx = nt * (NBLK // P) + m
            msl = slice(m * P, (m + 1) * P)
            xk_t = load_xk(nt)
            if half == 0:
                proj_ps[("v", nt, m)] = ps_p.tile([P, DG], F32, tag="p", name="psv")
            ps = proj_ps[("v", nt, m)]
            for k in range(4 * half, 4 * half + 4):
                nc.tensor.matmul(ps[:], xk_t[:, k, msl], wv_sb[:, k, :],
                                 start=(k == 0), stop=(k == KT - 1))
            if half == 1:
                del proj_ps[("v", nt, m)]
                v_row = v_aug[:, tidx]
                for h in range(HLOC):
                    nc.vector.tensor_add(
                        v_row[:, h * (hd + 1):h * (hd + 1) + hd],
                        ps[:, h * hd:(h + 1) * hd],
                        bv_bc[:, h * hd:(h + 1) * hd])
                # rowsum-trick ones columns, written here rather than by a
                # whole-tensor memset (see note at kTz memset)
                ones_view = v_row.rearrange("p (h e) -> p h e", e=hd + 1)[:, :, hd:hd + 1]
                nc.vector.memset(ones_view, 1.0)

        def emit_outproj(qt, jt):
            """One [128 t, 512 c] tile of the output projection."""
            qsl = slice(qt * P, (qt + 1) * P)
            jsl = slice(jt * NBLK, (jt + 1) * NBLK)
            ps = ps_p.tile([P, NBLK], F32, tag="p")
            for dt in range(MT):
                nc.tensor.matmul(ps[:], yT_sb[:, dt, qsl],
                                 wu_sb[:, dt, jsl],
                                 start=(dt == 0), stop=(dt == MT - 1))
            o_sb = outp.tile([P, NBLK], F32, tag="osb")
            nc.vector.tensor_copy(o_sb[:], ps[:])
            nc.sync.dma_start(out=out[qsl, jsl], in_=o_sb[:])

        # ---- filler stream: (n_matmuls, thunk) pairs, paced by budget ----
        filler = []

        def drain_filler(mm_budget):
            while mm_budget > 0 and filler:
                n_mm, fn = filler.pop(0)
                fn()
                mm_budget -= n_mm

        def queue_kv_block(nt):
            for m in range(MT):
                for half in range(2):
                    filler.append(
                        (4, lambda nt=nt, m=m, h=half: emit_kT(nt, m, h)))
            for m in range(NBLK // P):
                for half in range(2):
                    filler.append(
                        (4, lambda nt=nt, m=m, h=half: emit_v(nt, m, h)))

        def queue_qT(nt):
            for m in range(MT):
                for half in range(2):
                    filler.append(
                        (4, lambda nt=nt, m=m, h=half: emit_qT(nt, m, h)))

        def queue_outproj(qb):
            for qt in range(qb * NBLK // P, (qb + 1) * NBLK // P):
                for jt in range(C // NBLK):
                    filler.append(
                        (4, lambda qt=qt, jt=jt: emit_outproj(qt, jt)))

        def queue_v_block(nt):
            for m in range(NBLK // P):
                for half in range(2):
                    filler.append(
                        (4, lambda nt=nt, m=m, h=half: emit_v(nt, m, h)))

        def queue_kT(nt, m):
            for half in range(2):
                filler.append(
                    (4, lambda nt=nt, m=m, h=half: emit_kT(nt, m, h)))

        def queue_qT_m(nt, m):
            for half in range(2):
                filler.append(
                    (4, lambda nt=nt, m=m, h=half: emit_qT(nt, m, h)))

        # ---- prologue: only what S(0)/exp(0) strictly need ----
        for half in range(2):
            emit_qT(0, 0, half)
        for half in range(2):
            emit_kT(0, 0, half)

        # deadline-ordered filler: v blocks and remaining kT m0 slices pace
        # the first head-pair's k-sweep; later m-slices + qT(0) m-slices
        # before their head pairs start (hp reads kT m=hp, qT m=hp at
        # g = hp*16)
        queue_v_block(0)
        queue_kT(1, 0)
        queue_v_block(1)
        queue_kT(2, 0)
        queue_v_block(2)
        queue_kT(3, 0)
        queue_v_block(3)
        for m in range(1, MT):
            for nt in range(NB):
                queue_kT(nt, m)
            queue_qT_m(0, m)

        # ---- main attention loop ----
        # group g = (qb, hp, ktp, h2): S for 2 k-tiles of one head.
        groups = []
        for qb in range(NB):
            for hp in range(HP):
                for ktp in range(KTT // 2):
                    for h2 in range(2):
                        groups.append((qb, hp, ktp, h2))
        NG = len(groups)

        def emit_S(g):
            qb, hp, ktp, h2 = groups[g]
            h = 2 * hp + h2
            qsl = slice(qb * NBLK, (qb + 1) * NBLK)
            s = ps_s.tile([P, 2, NBLK], F32, tag="s", name="s_ps")
            for j in range(2):
                kt = 2 * ktp + j
                ksl = slice(kt * P, (kt + 1) * P)
                nc.tensor.matmul(s[:, j, :], kTz_sb[:, h, ksl],
                                 qT_sb[:, hp, qsl], start=True, stop=True)
            return s

        def emit_exp(g, s):
            p = ppool.tile([P, 2, NBLK], BF16, tag="p", name="p_sb")
            nc.scalar.activation(p[:], s[:], AF.Exp)
            return p

        o_tiles = {}   # (qb, h) -> psum tile accumulating [65, NBLK]
        norm_sbs = {}  # (qb, h) -> o_sb copy awaiting the batched recip
        rs_tiles = {}  # qb -> [HLOC, NBLK] gathered rowsums

        def emit_norm_copy(qb, h, o_t):
            """Evict o psum; gather the rowsum row into the per-qb batch
            (via DMA so the DVE queue stays clean)."""
            if qb not in rs_tiles:
                rs_tiles[qb] = epil.tile([HLOC, NBLK], F32, tag="rs",
                                         bufs=2, name="rs_all")
            o_sb = epil.tile([hd + 1, NBLK], F32, tag="o_sb", bufs=8,
                             name="o_sb")
            nc.vector.tensor_copy(o_sb[:], o_t[:])
            nc.sync.dma_start(out=rs_tiles[qb][h:h + 1, :],
                              in_=o_sb[hd:hd + 1, :])
            norm_sbs[(qb, h)] = o_sb

        def emit_norm_finish(qb):
            """One batched reciprocal for all 8 heads, then per-head
            broadcast + multiply into yT."""
            qsl = slice(qb * NBLK, (qb + 1) * NBLK)
            recip_t = epil.tile([HLOC, NBLK], F32, tag="recip", bufs=2,
                                name="recip_t")
            nc.vector.reciprocal(recip_t[:], rs_tiles.pop(qb)[:])
            for h in range(HLOC):
                hp, h2 = h // 2, h % 2
                hrow = slice(0, hd) if h2 == 0 else slice(hd, P)
                # partition_broadcast requires its source on partition 0
                rtmp = epil.tile([1, NBLK], F32, tag="rtmp", bufs=2,
                                 name="rtmp")
                nc.sync.dma_start(out=rtmp[:], in_=recip_t[h:h + 1, :])
                bcast = epil.tile([hd, NBLK], F32, tag="bcast", bufs=3,
                                  name="bcast")
                nc.gpsimd.partition_broadcast(bcast[:], rtmp[:])
                nc.vector.tensor_mul(yT_sb[hrow, hp, qsl],
                                     norm_sbs.pop((qb, h))[0:hd, :],
                                     bcast[:])

        def emit_V(g, p):
            qb, hp, ktp, h2 = groups[g]
            h = 2 * hp + h2
            key = (qb, h)
            if ktp == 0:
                o_tiles[key] = ps_o.tile([hd + 1, NBLK], F32, tag="o", name="o_ps")
            o_t = o_tiles[key]
            for j in range(2):
                kt = 2 * ktp + j
                nc.tensor.matmul(
                    o_t[:], v_aug[:, kt, h * (hd + 1):(h + 1) * (hd + 1)],
                    p[:, j, :], start=(kt == 0), stop=(kt == KTT - 1))
            if ktp == KTT // 2 - 1:
                emit_norm_copy(qb, h, o_tiles.pop(key))
                if hp == HP - 1 and h2 == 1:
                    emit_norm_finish(qb)

        # software pipeline: S(g+1) ahead of exp(g) ahead of V(g-1)
        s_tiles = {0: emit_S(0)}
        p_tiles = {}
        for g in range(NG):
            qb, hp, ktp, h2 = groups[g]
            if hp == 0 and ktp == 0 and h2 == 0:
                # q-block boundary: queue next q-block's Q projection and
                # the finished previous q-block's output projection
                if qb + 1 < NB:
                    queue_qT(qb + 1)
                if qb >= 1:
                    queue_outproj(qb - 1)
            if g + 1 < NG:
                s_tiles[g + 1] = emit_S(g + 1)
            p_tiles[g] = emit_exp(g, s_tiles.pop(g))
            if g - 1 >= 0:
                emit_V(g - 1, p_tiles.pop(g - 1))
            # pace the filler: the first k-sweep is production-bound on
            # the kT/v projections, so drain aggressively there; then keep
            # ~2 matmuls per group so the exp stream is never starved
            drain_filler(16 if g < 40 else (4 if g < 64 else 2))
        emit_V(NG - 1, p_tiles.pop(NG - 1))

        # ---- epilogue: last q-block's output projection + leftovers ----
        queue_outproj(NB - 1)
        drain_filler(10 ** 9)


    nc.compile()
    return nc


def _get_program():
    key = "main"
    if key not in _PROGRAM_CACHE:
        _PROGRAM_CACHE[key] = build_program()
    return _PROGRAM_CACHE[key]


def make_in_maps(x1, x2, Wq, bq, Wk, bk, Wv, bv, Wu, bu, n_cores=8):
    import ml_dtypes
    bf16 = ml_dtypes.bfloat16
    T, B, C = x1.shape
    H = 16
    DG = C // 2  # head-group feature dim (8 heads x 64)
    x1 = np.asarray(x1, np.float32)
    x2 = np.asarray(x2, np.float32)
    in_maps = []
    for core in range(n_cores):
        b, g = core // 2, core % 2
        gs = slice(g * DG, (g + 1) * DG)
        in_maps.append({
            "xqT": np.ascontiguousarray(x1[:, b, :].T).astype(bf16),
            "xkT": np.ascontiguousarray(x2[:, b, :].T).astype(bf16),
            "wqT": np.ascontiguousarray(np.asarray(Wq)[gs, :].T).astype(bf16),
            "wkT": np.ascontiguousarray(np.asarray(Wk)[gs, :].T).astype(bf16),
            "wvT": np.ascontiguousarray(np.asarray(Wv)[gs, :].T).astype(bf16),
            "wuT": np.ascontiguousarray(np.asarray(Wu)[:, gs].T).astype(bf16),
            "bq": np.ascontiguousarray(
                np.asarray(bq, np.float32)[gs].reshape(-1, 128).T),
            "bk": np.ascontiguousarray(
                np.asarray(bk, np.float32)[gs].reshape(-1, 128).T),
            "bv": np.asarray(bv, np.float32)[gs].reshape(1, DG),
        })
    return in_maps


def kernel(x1, x2, Wq, bq, Wk, bk, Wv, bv, Wu, bu, _results_hook=None):
    _, _, _, _, run_bass_kernel_spmd = _imports()
    T, B, C = x1.shape
    nc = _get_program()
    in_maps = make_in_maps(x1, x2, Wq, bq, Wk, bk, Wv, bv, Wu, bu)
    br = run_bass_kernel_spmd(nc, in_maps, list(range(8)))
    if _results_hook is not None:
        _results_hook(br)
    outs = [np.asarray(r["out"], np.float32) for r in br.results]
    bu = np.asarray(bu, np.float32)
    full = np.stack([outs[2 * b] + outs[2 * b + 1] for b in range(B)], axis=0)
    full += bu.reshape(1, 1, -1)
    return full.astype(np.float32)
